# revision 27
# baseline (speedup 1.0000x reference)
"""DotProductPredictor edge-score kernel for 8 TRN2 NeuronCores.

score[e] = sigmoid(dot(features[src[e]], features[dst[e]]))

Strategy (self-contained; shapes hardcoded):
  - Shard the 1.2M edges across 8 cores with a balanced global deal: edges
    are bucketed into 16 (src_bucket, dst_bucket) groups (4 ranges of 25000
    node ids each; dma_gather's int16 indices only address <=32768 rows) and
    each group's edges are dealt round-robin over the 8 cores, so every
    (core, group) cell has nearly identical size and one padded capacity G
    lets all 8 cores share a single compiled program.
  - features are cast to fp16 on host and laid out as a 256B-strided padded
    table [100000, 128] (64 fp16 payload + 64 zeros). The GPSIMD dma_gather
    ucode encodes the row stride in 256B units but the element size in
    bytes, so each gather descriptor moves only the 128B payload - half the
    HBM traffic of the f32 version. (bass' dma_gather wrapper over-asserts
    elem_size_bytes % 256 == 0; that restriction only applies to the
    transpose path in the ucode, so _dma_gather_raw below mirrors the
    wrapper without it.)
  - On device per group: gather src rows and dst rows (multi-packet SWDGE,
    one 9472-index instruction each; group g runs on SWDGE queue g%4 so the
    4 Q7 queue-pairs generate descriptors concurrently - measured 3.6x
    faster than one queue; per-group valid counts come from registers so
    padding costs nothing), multiply elementwise on the vector engine (fp16
    2x packed mode), segmented reduce_sum into an f32 accumulator, sigmoid
    on the scalar engine, and write the fp16 [128, T/128] score block once.
    4 h-tile buffers are needed to fully hide the DVE work under the
    gathers (bufs=3 measured +220us/pass).
  - Measured cost structure (paired repeat-slope on HW): the kernel is
    bound by per-descriptor work on the Q7 pairs (~9ns/descriptor/pair:
    index read + descriptor gen), NOT by HBM bytes - halving descriptors
    halved time while doubling bytes cost only +15%. Hence: one descriptor
    per edge endpoint is the floor of this design, and the fp16 row payload
    (vs f32) mostly helps by halving SBUF pressure (enabling bufs=4) and
    the table upload, not via bandwidth.
  - Host unpads/unsorts scores back to original edge order. Group overflow
    (statistically ~never: capacity is ~3 sigma above the max cell size)
    is computed on host as a fallback, as is any device failure.
"""
import numpy as np

N_NODES = 100000
N_EDGES = 1200000
D = 64
NC = 8
BUCKET = 25000               # node-id range per bucket (4 * 25000 = 100000)
NB = 4
NGRP = NB * NB               # 16 groups
G = 9472                     # padded edges per (core, group); mean 9375, sigma ~33
T = NGRP * G                 # 151552 padded edges per core
PADROW = 128                 # fp16 elements per padded table row (256B stride)
SCRATCH = 65536              # dynamic DMA scratch bytes/partition (descriptor rings)
NQDEF = 4                    # SWDGE queues; group g runs on queue g % NQDEF

_CACHE = {}


def _dma_gather_raw(eng, out_ap, in_ap, idxs_ap, num_idxs, num_idxs_reg,
                    elem_size, elem_step, queue_num, single_packet=False):
    """nc.gpsimd.dma_gather for the non-transpose HBM path, without the
    elem_size_bytes % 256 assert (ucode only requires that for transpose;
    the row *stride* is what must be a multiple of 256B)."""
    from concourse import mybir
    from concourse import ap_utils
    from concourse.bass import MemorySpace

    eng._assert_queue_num(queue_num)
    assert idxs_ap.dtype == mybir.dt.int16
    assert in_ap.dtype == out_ap.dtype
    assert in_ap.space == MemorySpace.DRAM
    assert idxs_ap.space == MemorySpace.SBUF
    assert out_ap.space == MemorySpace.SBUF
    assert ap_utils.ap_is_contiguous(out_ap.ap[1:])
    assert ap_utils.ap_is_contiguous(idxs_ap.ap[1:])
    assert in_ap.ap[-1][1] == out_ap.ap[-1][1] == elem_size
    assert out_ap.ap[0][1] * out_ap.ap[1][1] == ((num_idxs + 127) // 128) * 128
    assert in_ap.ap[0][0] == elem_step
    stride_bytes = elem_step * mybir.dt.size(in_ap.dtype)
    stride_bytes_256 = stride_bytes // 256
    assert stride_bytes_256 * 256 == stride_bytes and 0 < stride_bytes_256 < 256

    _in_ap = eng.lower_ap_dma(in_ap, for_custom_bir_dma=True)
    _idxs_ap = eng.lower_ap(idxs_ap)
    _out_ap = eng.lower_ap(out_ap)
    return eng.add_instruction(
        mybir.InstDMAGatherAnt(
            name=eng.bass.get_next_instruction_name(),
            ins=[*_in_ap, _idxs_ap, eng.lower_val_access(eng.to_reg(num_idxs_reg))],
            outs=[_out_ap],
            transpose=False,
            num_idxs=num_idxs,
            elem_size=elem_size,
            stride_bytes_256=stride_bytes_256,
            gen_mode=0,
            single_packet=single_packet,
            queue_num=queue_num,
            sbuf_tokens_per_rank=0,
            sbuf_free_dim_per_rank=0,
            sbuf_free_dim_pad_per_rank=0,
            sbuf_byte_offset=0,
        )
    )


def _reduce_add_raw(eng, out_ap, in_ap):
    """vector.tensor_reduce(op=add, axis=X) without the low-precision-output
    guard (the DVE accumulates fine for our 8-element partial sums; the f32
    second stage restores precision)."""
    from concourse import mybir

    return eng.add_instruction(
        mybir.InstTensorReduce(
            name=f"I-{eng.bass.next_id()}",
            op=mybir.AluOpType.add,
            axis=mybir.AxisListType.X,
            ins=[eng.lower_ap(in_ap.opt(keep_dims={0, len(in_ap.shape) - 1}), opt=False)],
            outs=[eng.lower_ap(out_ap)],
            apply_absolute_value=None,
            apply_transpose=None,
            negate=None,
        )
    )


def _build_program():
    import os
    import concourse.tile as tile
    from concourse import bacc, mybir

    skip_gather = os.environ.get("KERNEL_SKIP_GATHER") == "1"
    skip_compute = os.environ.get("KERNEL_SKIP_COMPUTE") == "1"
    single_packet = os.environ.get("KERNEL_SP") == "1"
    halfdesc = os.environ.get("KERNEL_HALFDESC") == "1"  # timing probe only
    red2 = os.environ.get("KERNEL_RED2") == "1"
    nq = int(os.environ.get("KERNEL_NQ", str(NQDEF)))
    hbufs = int(os.environ.get("KERNEL_BUFS", "4"))
    nrep = int(os.environ.get("KERNEL_REPEAT", "1"))
    if halfdesc:
        skip_compute = True

    nc = bacc.Bacc(
        "TRN2",
        target_bir_lowering=False,
        debug=False,
        num_devices=NC,
        dynamic_dma_scratch_size=SCRATCH,
        num_swdge_queues=max(nq, 1),
    )
    gc16 = G // 16               # idx columns per group
    feat = nc.dram_tensor("featpad", [N_NODES, PADROW], mybir.dt.float16, kind="ExternalInput").ap()
    idx_s = nc.dram_tensor("idx_s", [128, T // 16], mybir.dt.int16, kind="ExternalInput").ap()
    idx_d = nc.dram_tensor("idx_d", [128, T // 16], mybir.dt.int16, kind="ExternalInput").ap()
    counts = nc.dram_tensor("counts", [1, 128], mybir.dt.int32, kind="ExternalInput").ap()
    out = nc.dram_tensor("scores", [128, T // 128], mybir.dt.float16, kind="ExternalOutput").ap()

    cols = G // 128              # 74 columns per group block

    with tile.TileContext(nc) as tc:
        with (
            tc.tile_pool(name="idx", bufs=1) as idxp,
            tc.tile_pool(name="acc", bufs=1) as accp,
            tc.tile_pool(name="h", bufs=hbufs) as hp,
        ):
            ia = idxp.tile([128, T // 16], mybir.dt.int16, tag="ia")
            ib = idxp.tile([128, T // 16], mybir.dt.int16, tag="ib")
            cnt = idxp.tile([1, 128], mybir.dt.int32, tag="cnt")
            nc.sync.dma_start(out=cnt[:], in_=counts)
            nc.sync.dma_start(out=ia[:], in_=idx_s)
            nc.sync.dma_start(out=ib[:], in_=idx_d)
            reg = nc.gpsimd.alloc_register("cnt_g")

            acc = accp.tile([128, T // 128], mybir.dt.float32, tag="acc")
            sig = accp.tile([128, T // 128], mybir.dt.float16, tag="sig")
            if skip_compute:
                nc.vector.memset(acc[:], 0.0)

            for rep in range(nrep):
                for g in range(NGRP):
                    bs, bd = divmod(g, NB)
                    q = g % max(nq, 1)
                    hu = hp.tile([128, cols * D], mybir.dt.float16, tag="hu")
                    hv = hp.tile([128, cols * D], mybir.dt.float16, tag="hv")
                    if skip_gather:
                        nc.vector.memset(hu[:], 0.125)
                        nc.vector.memset(hv[:], 0.25)
                    elif halfdesc:
                        # timing probe: half the descriptors, same bytes
                        # (reads the full 256B padded row per index)
                        nc.gpsimd.reg_load(reg, cnt[0:1, g : g + 1])
                        _dma_gather_raw(
                            nc.gpsimd,
                            hu[:].rearrange("p (c d) -> p c d", d=PADROW),
                            feat[bs * BUCKET : (bs + 1) * BUCKET, 0:PADROW],
                            ia[:, g * gc16 : g * gc16 + (G // 2) // 16],
                            G // 2, reg, PADROW, PADROW, q, single_packet,
                        )
                        _dma_gather_raw(
                            nc.gpsimd,
                            hv[:].rearrange("p (c d) -> p c d", d=PADROW),
                            feat[bd * BUCKET : (bd + 1) * BUCKET, 0:PADROW],
                            ib[:, g * gc16 : g * gc16 + (G // 2) // 16],
                            G // 2, reg, PADROW, PADROW, q, single_packet,
                        )
                    else:
                        nc.gpsimd.reg_load(reg, cnt[0:1, g : g + 1])
                        _dma_gather_raw(
                            nc.gpsimd,
                            hu[:].rearrange("p (c d) -> p c d", d=D),
                            feat[bs * BUCKET : (bs + 1) * BUCKET, 0:D],
                            ia[:, g * gc16 : (g + 1) * gc16],
                            G, reg, D, PADROW, q, single_packet,
                        )
                        _dma_gather_raw(
                            nc.gpsimd,
                            hv[:].rearrange("p (c d) -> p c d", d=D),
                            feat[bd * BUCKET : (bd + 1) * BUCKET, 0:D],
                            ib[:, g * gc16 : (g + 1) * gc16],
                            G, reg, D, PADROW, q, single_packet,
                        )
                    if not skip_compute:
                        nc.vector.tensor_tensor(
                            out=hu[:], in0=hu[:], in1=hv[:], op=mybir.AluOpType.mult,
                        )
                        if red2:
                            # two-stage reduce keeps stage 1 all-fp16 (DVE 2x
                            # packed mode); stage 2 accumulates in f32
                            _reduce_add_raw(
                                nc.vector,
                                hv[:, : cols * 8],
                                hu[:].rearrange("p (c a b) -> p c a b", a=8, b=8),
                            )
                            nc.vector.reduce_sum(
                                out=acc[:, g * cols : (g + 1) * cols],
                                in_=hv[:, : cols * 8].rearrange("p (c a) -> p c a", a=8),
                                axis=mybir.AxisListType.X,
                            )
                        else:
                            nc.vector.reduce_sum(
                                out=acc[:, g * cols : (g + 1) * cols],
                                in_=hu[:].rearrange("p (c d) -> p c d", d=D),
                                axis=mybir.AxisListType.X,
                            )

            nc.scalar.activation(sig[:], acc[:], mybir.ActivationFunctionType.Sigmoid)
            nc.sync.dma_start(out=out, in_=sig[:])

    nc.compile()
    return nc


def _prep_all(features, src64, dst64):
    """Host layout: deal each of the 16 groups' edges round-robin over the 8
    cores, build per-core padded bucket-local int16 index arrays (wrapped
    into dma_gather's 16-partition layout), per-(core,group) valid counts,
    and the mapping back to original edge order.

    Returns (in_maps, core_of, flatpos, spill_ids, featpad16).
      core_of[e]: which core computes edge e (-1 if spilled to host)
      flatpos[e]: padded position of edge e within its core's T-vector
    """
    featpad = np.zeros((N_NODES, PADROW), dtype=np.float16)
    featpad[:, :D] = features.astype(np.float16)

    grp = (src64 // BUCKET) * NB + (dst64 // BUCKET)     # [E]
    order = np.argsort(grp, kind="stable")
    sizes = np.bincount(grp, minlength=NGRP)
    starts = np.zeros(NGRP, dtype=np.int64)
    np.cumsum(sizes[:-1], out=starts[1:])

    core_of = np.empty(N_EDGES, dtype=np.int8)
    flatpos = np.empty(N_EDGES, dtype=np.int64)
    s_pad = np.full((NC, T), -1, dtype=np.int16)
    d_pad = np.full((NC, T), -1, dtype=np.int16)
    counts = np.zeros((NC, 128), dtype=np.int32)
    spill = []
    import os
    sort_cell = os.environ.get("KERNEL_SORT", "0") == "1"
    for g in range(NGRP):
        members = order[starts[g] : starts[g] + sizes[g]]
        j = np.arange(members.shape[0], dtype=np.int64)
        core = (j % NC).astype(np.int8)
        ok = (j // NC) < G
        if not ok.all():
            spill.append(members[~ok])
            members, core = members[ok], core[ok]
        sl = (src64[members] - (g // NB) * BUCKET).astype(np.int16)
        dl = (dst64[members] - (g % NB) * BUCKET).astype(np.int16)
        for c in range(NC):
            m = core == c
            k = int(m.sum())
            mem_c, sl_c, dl_c = members[m], sl[m], dl[m]
            if sort_cell:
                # ascending src ids -> consecutive gather descriptors hit
                # nearby HBM rows (DRAM row-buffer locality)
                o = np.argsort(sl_c, kind="stable")
                mem_c, sl_c, dl_c = mem_c[o], sl_c[o], dl_c[o]
            s_pad[c, g * G : g * G + k] = sl_c
            d_pad[c, g * G : g * G + k] = dl_c
            core_of[mem_c] = c
            flatpos[mem_c] = g * G + np.arange(k, dtype=np.int64)
            v2 = max(k, 128)
            if v2 > k:
                s_pad[c, g * G + k : g * G + v2] = 0
                d_pad[c, g * G + k : g * G + v2] = 0
            counts[c, g] = v2
    spill_ids = np.concatenate(spill) if spill else np.zeros(0, dtype=np.int64)
    core_of[spill_ids] = -1
    flatpos[spill_ids] = -1

    def wrap(arr):
        w = arr.reshape(T // 16, 16).T           # [16, T/16]
        return np.ascontiguousarray(np.tile(w, (8, 1)))  # [128, T/16]

    in_maps = []
    for c in range(NC):
        in_maps.append({
            "featpad": featpad,
            "idx_s": wrap(s_pad[c]),
            "idx_d": wrap(d_pad[c]),
            "counts": counts[c].reshape(1, 128),
        })
    return in_maps, core_of, flatpos, spill_ids, featpad


def _host_scores(features, s, d):
    sc = np.einsum("ij,ij->i", features[s], features[d], dtype=np.float32)
    return (1.0 / (1.0 + np.exp(-sc))).astype(np.float32)


def kernel(features, src, dst):
    from concourse.bass_utils import run_bass_kernel_spmd

    features = np.asarray(features, dtype=np.float32)
    src64 = np.asarray(src).astype(np.int64)
    dst64 = np.asarray(dst).astype(np.int64)

    if features.shape != (N_NODES, D) or src64.shape != (N_EDGES,) or dst64.shape != (N_EDGES,):
        return _host_scores(features, src64, dst64)

    if "nc" not in _CACHE:
        _CACHE["nc"] = _build_program()
    nc = _CACHE["nc"]

    in_maps, core_of, flatpos, spill_ids, _ = _prep_all(features, src64, dst64)

    try:
        res = run_bass_kernel_spmd(nc, in_maps, list(range(NC))).results
    except Exception:
        # device failure: fall back to a correct host computation
        return _host_scores(features, src64, dst64)

    rng = np.random.default_rng(12345)
    out = np.empty(N_EDGES, dtype=np.float32)
    eids = np.arange(N_EDGES, dtype=np.int64)
    for c in range(NC):
        scores_pad = res[c]["scores"].T.ravel().astype(np.float32)  # padded pos -> score
        m = core_of == c
        ids = eids[m]
        out[ids] = scores_pad[flatpos[ids]]
        # cheap integrity check on a random sample; recompute on host if the
        # device result is corrupt (defends against rare SWDGE ring races)
        probe = rng.choice(ids, size=min(2048, ids.size), replace=False)
        want = _host_scores(features, src64[probe], dst64[probe])
        if not np.allclose(out[probe], want, rtol=2e-2, atol=5e-3):
            out[ids] = _host_scores(features, src64[ids], dst64[ids])
    if spill_ids.size:
        out[spill_ids] = _host_scores(features, src64[spill_ids], dst64[spill_ids])
    return out


# revision 32
# speedup vs baseline: 1.0485x; 1.0485x over previous
"""DotProductPredictor edge-score kernel for 8 TRN2 NeuronCores.

score[e] = sigmoid(dot(features[src[e]], features[dst[e]]))

Strategy (self-contained; shapes hardcoded):
  - Shard the 1.2M edges across 8 cores with a balanced global deal: edges
    are bucketed into 16 (src_bucket, dst_bucket) groups (4 ranges of 25000
    node ids each; dma_gather's int16 indices only address <=32768 rows) and
    each group's edges are dealt round-robin over the 8 cores, so every
    (core, group) cell has nearly identical size and one padded capacity G
    lets all 8 cores share a single compiled program.
  - features are cast to fp16 on host and laid out as a 256B-strided padded
    table [100000, 128] (64 fp16 payload + 64 zeros). The GPSIMD dma_gather
    ucode encodes the row stride in 256B units but the element size in
    bytes, so each gather descriptor moves only the 128B payload - half the
    HBM traffic of the f32 version. (bass' dma_gather wrapper over-asserts
    elem_size_bytes % 256 == 0; that restriction only applies to the
    transpose path in the ucode, so _dma_gather_raw below mirrors the
    wrapper without it.)
  - On device per group: gather src rows and dst rows (multi-packet SWDGE,
    one 9472-index instruction each; group g runs on SWDGE queue g%4 so the
    4 Q7 queue-pairs generate descriptors concurrently - measured 3.6x
    faster than one queue; per-group valid counts come from registers so
    padding costs nothing), multiply elementwise on the vector engine (fp16
    2x packed mode), segmented reduce_sum into an f32 accumulator, sigmoid
    per group on the scalar engine, and write each group's fp16 score block
    back as it completes. 6 h-tile buffers pipeline deep enough to hide the
    DVE work and buffer-release stalls under the gathers (bufs=3 measured
    +220us/pass vs 4; 5-6 another ~-35us).
  - Measured cost structure (paired repeat-slope on HW): the kernel is
    bound by per-descriptor work on the Q7 pairs (~9ns/descriptor/pair:
    index read + descriptor gen), NOT by HBM bytes - halving descriptors
    halved time while doubling bytes cost only +15%. Hence: one descriptor
    per edge endpoint is the floor of this design, and the fp16 row payload
    (vs f32) mostly helps by halving SBUF pressure (enabling bufs=4) and
    the table upload, not via bandwidth.
  - Host unpads/unsorts scores back to original edge order. Group overflow
    (statistically ~never: capacity is ~3 sigma above the max cell size)
    is computed on host as a fallback, as is any device failure.
"""
import numpy as np

N_NODES = 100000
N_EDGES = 1200000
D = 64
NC = 8
BUCKET = 25000               # node-id range per bucket (4 * 25000 = 100000)
NB = 4
NGRP = NB * NB               # 16 groups
G = 9472                     # padded edges per (core, group); mean 9375, sigma ~33
T = NGRP * G                 # 151552 padded edges per core
PADROW = 128                 # fp16 elements per padded table row (256B stride)
SCRATCH = 65536              # dynamic DMA scratch bytes/partition (descriptor rings)
NQDEF = 4                    # SWDGE queues; group g runs on queue g % NQDEF

_CACHE = {}


def _dma_gather_raw(eng, out_ap, in_ap, idxs_ap, num_idxs, num_idxs_reg,
                    elem_size, elem_step, queue_num, single_packet=False):
    """nc.gpsimd.dma_gather for the non-transpose HBM path, without the
    elem_size_bytes % 256 assert (ucode only requires that for transpose;
    the row *stride* is what must be a multiple of 256B)."""
    from concourse import mybir
    from concourse import ap_utils
    from concourse.bass import MemorySpace

    eng._assert_queue_num(queue_num)
    assert idxs_ap.dtype == mybir.dt.int16
    assert in_ap.dtype == out_ap.dtype
    assert in_ap.space == MemorySpace.DRAM
    assert idxs_ap.space == MemorySpace.SBUF
    assert out_ap.space == MemorySpace.SBUF
    assert ap_utils.ap_is_contiguous(out_ap.ap[1:])
    assert ap_utils.ap_is_contiguous(idxs_ap.ap[1:])
    assert in_ap.ap[-1][1] == out_ap.ap[-1][1] == elem_size
    assert out_ap.ap[0][1] * out_ap.ap[1][1] == ((num_idxs + 127) // 128) * 128
    assert in_ap.ap[0][0] == elem_step
    stride_bytes = elem_step * mybir.dt.size(in_ap.dtype)
    stride_bytes_256 = stride_bytes // 256
    assert stride_bytes_256 * 256 == stride_bytes and 0 < stride_bytes_256 < 256

    _in_ap = eng.lower_ap_dma(in_ap, for_custom_bir_dma=True)
    _idxs_ap = eng.lower_ap(idxs_ap)
    _out_ap = eng.lower_ap(out_ap)
    return eng.add_instruction(
        mybir.InstDMAGatherAnt(
            name=eng.bass.get_next_instruction_name(),
            ins=[*_in_ap, _idxs_ap, eng.lower_val_access(eng.to_reg(num_idxs_reg))],
            outs=[_out_ap],
            transpose=False,
            num_idxs=num_idxs,
            elem_size=elem_size,
            stride_bytes_256=stride_bytes_256,
            gen_mode=0,
            single_packet=single_packet,
            queue_num=queue_num,
            sbuf_tokens_per_rank=0,
            sbuf_free_dim_per_rank=0,
            sbuf_free_dim_pad_per_rank=0,
            sbuf_byte_offset=0,
        )
    )


def _reduce_add_raw(eng, out_ap, in_ap):
    """vector.tensor_reduce(op=add, axis=X) without the low-precision-output
    guard (the DVE accumulates fine for our 8-element partial sums; the f32
    second stage restores precision)."""
    from concourse import mybir

    return eng.add_instruction(
        mybir.InstTensorReduce(
            name=f"I-{eng.bass.next_id()}",
            op=mybir.AluOpType.add,
            axis=mybir.AxisListType.X,
            ins=[eng.lower_ap(in_ap.opt(keep_dims={0, len(in_ap.shape) - 1}), opt=False)],
            outs=[eng.lower_ap(out_ap)],
            apply_absolute_value=None,
            apply_transpose=None,
            negate=None,
        )
    )


def _build_program():
    import os
    import concourse.tile as tile
    from concourse import bacc, mybir

    skip_gather = os.environ.get("KERNEL_SKIP_GATHER") == "1"
    skip_compute = os.environ.get("KERNEL_SKIP_COMPUTE") == "1"
    single_packet = os.environ.get("KERNEL_SP") == "1"
    halfdesc = os.environ.get("KERNEL_HALFDESC") == "1"  # timing probe only
    red2 = os.environ.get("KERNEL_RED2") == "1"
    nosplit = os.environ.get("KERNEL_NOSPLIT") == "1"
    nq = int(os.environ.get("KERNEL_NQ", str(NQDEF)))
    hbufs = int(os.environ.get("KERNEL_BUFS", "6"))
    nrep = int(os.environ.get("KERNEL_REPEAT", "1"))
    if halfdesc:
        skip_compute = True

    nc = bacc.Bacc(
        "TRN2",
        target_bir_lowering=False,
        debug=False,
        num_devices=NC,
        dynamic_dma_scratch_size=SCRATCH,
        num_swdge_queues=max(nq, 1),
    )
    gc16 = G // 16               # idx columns per group
    feat = nc.dram_tensor("featpad", [N_NODES, PADROW], mybir.dt.float16, kind="ExternalInput").ap()
    idx_s = nc.dram_tensor("idx_s", [128, T // 16], mybir.dt.int16, kind="ExternalInput").ap()
    idx_d = nc.dram_tensor("idx_d", [128, T // 16], mybir.dt.int16, kind="ExternalInput").ap()
    counts = nc.dram_tensor("counts", [1, 128], mybir.dt.int32, kind="ExternalInput").ap()
    out = nc.dram_tensor("scores", [128, T // 128], mybir.dt.float16, kind="ExternalOutput").ap()

    cols = G // 128              # 74 columns per group block

    with tile.TileContext(nc) as tc:
        with (
            tc.tile_pool(name="idx", bufs=1) as idxp,
            tc.tile_pool(name="acc", bufs=1) as accp,
            tc.tile_pool(name="h", bufs=hbufs) as hp,
        ):
            ia = idxp.tile([128, T // 16], mybir.dt.int16, tag="ia")
            ib = idxp.tile([128, T // 16], mybir.dt.int16, tag="ib")
            cnt = idxp.tile([1, 128], mybir.dt.int32, tag="cnt")
            nc.sync.dma_start(out=cnt[:], in_=counts)
            if nosplit:
                nc.sync.dma_start(out=ia[:], in_=idx_s)
                nc.sync.dma_start(out=ib[:], in_=idx_d)
            else:
                # per-group slices: group 0's gathers only wait for their own
                # index columns, not the full 2x2.4MB load
                for g in range(NGRP):
                    sl = slice(g * gc16, (g + 1) * gc16)
                    nc.sync.dma_start(out=ia[:, sl], in_=idx_s[:, sl])
                    nc.sync.dma_start(out=ib[:, sl], in_=idx_d[:, sl])
            reg = nc.gpsimd.alloc_register("cnt_g")

            acc = accp.tile([128, T // 128], mybir.dt.float32, tag="acc")
            sig = accp.tile([128, T // 128], mybir.dt.float16, tag="sig")
            if skip_compute:
                nc.vector.memset(acc[:], 0.0)

            for rep in range(nrep):
                for g in range(NGRP):
                    bs, bd = divmod(g, NB)
                    q = g % max(nq, 1)
                    hu = hp.tile([128, cols * D], mybir.dt.float16, tag="hu")
                    hv = hp.tile([128, cols * D], mybir.dt.float16, tag="hv")
                    if skip_gather:
                        nc.vector.memset(hu[:], 0.125)
                        nc.vector.memset(hv[:], 0.25)
                    elif halfdesc:
                        # timing probe: half the descriptors, same bytes
                        # (reads the full 256B padded row per index)
                        nc.gpsimd.reg_load(reg, cnt[0:1, g : g + 1])
                        _dma_gather_raw(
                            nc.gpsimd,
                            hu[:].rearrange("p (c d) -> p c d", d=PADROW),
                            feat[bs * BUCKET : (bs + 1) * BUCKET, 0:PADROW],
                            ia[:, g * gc16 : g * gc16 + (G // 2) // 16],
                            G // 2, reg, PADROW, PADROW, q, single_packet,
                        )
                        _dma_gather_raw(
                            nc.gpsimd,
                            hv[:].rearrange("p (c d) -> p c d", d=PADROW),
                            feat[bd * BUCKET : (bd + 1) * BUCKET, 0:PADROW],
                            ib[:, g * gc16 : g * gc16 + (G // 2) // 16],
                            G // 2, reg, PADROW, PADROW, q, single_packet,
                        )
                    else:
                        nc.gpsimd.reg_load(reg, cnt[0:1, g : g + 1])
                        _dma_gather_raw(
                            nc.gpsimd,
                            hu[:].rearrange("p (c d) -> p c d", d=D),
                            feat[bs * BUCKET : (bs + 1) * BUCKET, 0:D],
                            ia[:, g * gc16 : (g + 1) * gc16],
                            G, reg, D, PADROW, q, single_packet,
                        )
                        _dma_gather_raw(
                            nc.gpsimd,
                            hv[:].rearrange("p (c d) -> p c d", d=D),
                            feat[bd * BUCKET : (bd + 1) * BUCKET, 0:D],
                            ib[:, g * gc16 : (g + 1) * gc16],
                            G, reg, D, PADROW, q, single_packet,
                        )
                    if not skip_compute:
                        nc.vector.tensor_tensor(
                            out=hu[:], in0=hu[:], in1=hv[:], op=mybir.AluOpType.mult,
                        )
                        if red2:
                            # two-stage reduce keeps stage 1 all-fp16 (DVE 2x
                            # packed mode); stage 2 accumulates in f32
                            _reduce_add_raw(
                                nc.vector,
                                hv[:, : cols * 8],
                                hu[:].rearrange("p (c a b) -> p c a b", a=8, b=8),
                            )
                            nc.vector.reduce_sum(
                                out=acc[:, g * cols : (g + 1) * cols],
                                in_=hv[:, : cols * 8].rearrange("p (c a) -> p c a", a=8),
                                axis=mybir.AxisListType.X,
                            )
                        else:
                            nc.vector.reduce_sum(
                                out=acc[:, g * cols : (g + 1) * cols],
                                in_=hu[:].rearrange("p (c d) -> p c d", d=D),
                                axis=mybir.AxisListType.X,
                            )
                        if not nosplit:
                            # per-group tail: sigmoid + writeback pipeline
                            # behind the remaining groups' gathers
                            gs = slice(g * cols, (g + 1) * cols)
                            nc.scalar.activation(
                                sig[:, gs], acc[:, gs],
                                mybir.ActivationFunctionType.Sigmoid,
                            )
                            nc.sync.dma_start(out=out[:, gs], in_=sig[:, gs])

            if nosplit or skip_compute:
                nc.scalar.activation(sig[:], acc[:], mybir.ActivationFunctionType.Sigmoid)
                nc.sync.dma_start(out=out, in_=sig[:])

    nc.compile()
    return nc


def _prep_all(features, src64, dst64):
    """Host layout: deal each of the 16 groups' edges round-robin over the 8
    cores, build per-core padded bucket-local int16 index arrays (wrapped
    into dma_gather's 16-partition layout), per-(core,group) valid counts,
    and the mapping back to original edge order.

    Returns (in_maps, core_of, flatpos, spill_ids, featpad16).
      core_of[e]: which core computes edge e (-1 if spilled to host)
      flatpos[e]: padded position of edge e within its core's T-vector
    """
    featpad = np.zeros((N_NODES, PADROW), dtype=np.float16)
    featpad[:, :D] = features.astype(np.float16)

    grp = (src64 // BUCKET) * NB + (dst64 // BUCKET)     # [E]
    order = np.argsort(grp, kind="stable")
    sizes = np.bincount(grp, minlength=NGRP)
    starts = np.zeros(NGRP, dtype=np.int64)
    np.cumsum(sizes[:-1], out=starts[1:])

    core_of = np.empty(N_EDGES, dtype=np.int8)
    flatpos = np.empty(N_EDGES, dtype=np.int64)
    s_pad = np.full((NC, T), -1, dtype=np.int16)
    d_pad = np.full((NC, T), -1, dtype=np.int16)
    counts = np.zeros((NC, 128), dtype=np.int32)
    spill = []
    import os
    sort_cell = os.environ.get("KERNEL_SORT", "0") == "1"
    for g in range(NGRP):
        members = order[starts[g] : starts[g] + sizes[g]]
        j = np.arange(members.shape[0], dtype=np.int64)
        core = (j % NC).astype(np.int8)
        ok = (j // NC) < G
        if not ok.all():
            spill.append(members[~ok])
            members, core = members[ok], core[ok]
        sl = (src64[members] - (g // NB) * BUCKET).astype(np.int16)
        dl = (dst64[members] - (g % NB) * BUCKET).astype(np.int16)
        for c in range(NC):
            m = core == c
            k = int(m.sum())
            mem_c, sl_c, dl_c = members[m], sl[m], dl[m]
            if sort_cell:
                # ascending src ids -> consecutive gather descriptors hit
                # nearby HBM rows (DRAM row-buffer locality)
                o = np.argsort(sl_c, kind="stable")
                mem_c, sl_c, dl_c = mem_c[o], sl_c[o], dl_c[o]
            s_pad[c, g * G : g * G + k] = sl_c
            d_pad[c, g * G : g * G + k] = dl_c
            core_of[mem_c] = c
            flatpos[mem_c] = g * G + np.arange(k, dtype=np.int64)
            v2 = max(k, 128)
            if v2 > k:
                s_pad[c, g * G + k : g * G + v2] = 0
                d_pad[c, g * G + k : g * G + v2] = 0
            counts[c, g] = v2
    spill_ids = np.concatenate(spill) if spill else np.zeros(0, dtype=np.int64)
    core_of[spill_ids] = -1
    flatpos[spill_ids] = -1

    def wrap(arr):
        w = arr.reshape(T // 16, 16).T           # [16, T/16]
        return np.ascontiguousarray(np.tile(w, (8, 1)))  # [128, T/16]

    in_maps = []
    for c in range(NC):
        in_maps.append({
            "featpad": featpad,
            "idx_s": wrap(s_pad[c]),
            "idx_d": wrap(d_pad[c]),
            "counts": counts[c].reshape(1, 128),
        })
    return in_maps, core_of, flatpos, spill_ids, featpad


def _host_scores(features, s, d):
    sc = np.einsum("ij,ij->i", features[s], features[d], dtype=np.float32)
    return (1.0 / (1.0 + np.exp(-sc))).astype(np.float32)


def kernel(features, src, dst):
    from concourse.bass_utils import run_bass_kernel_spmd

    features = np.asarray(features, dtype=np.float32)
    src64 = np.asarray(src).astype(np.int64)
    dst64 = np.asarray(dst).astype(np.int64)

    if features.shape != (N_NODES, D) or src64.shape != (N_EDGES,) or dst64.shape != (N_EDGES,):
        return _host_scores(features, src64, dst64)

    if "nc" not in _CACHE:
        _CACHE["nc"] = _build_program()
    nc = _CACHE["nc"]

    in_maps, core_of, flatpos, spill_ids, _ = _prep_all(features, src64, dst64)

    try:
        res = run_bass_kernel_spmd(nc, in_maps, list(range(NC))).results
    except Exception:
        # device failure: fall back to a correct host computation
        return _host_scores(features, src64, dst64)

    rng = np.random.default_rng(12345)
    out = np.empty(N_EDGES, dtype=np.float32)
    eids = np.arange(N_EDGES, dtype=np.int64)
    for c in range(NC):
        scores_pad = res[c]["scores"].T.ravel().astype(np.float32)  # padded pos -> score
        m = core_of == c
        ids = eids[m]
        out[ids] = scores_pad[flatpos[ids]]
        # cheap integrity check on a random sample; recompute on host if the
        # device result is corrupt (defends against rare SWDGE ring races)
        probe = rng.choice(ids, size=min(2048, ids.size), replace=False)
        want = _host_scores(features, src64[probe], dst64[probe])
        if not np.allclose(out[probe], want, rtol=2e-2, atol=5e-3):
            out[ids] = _host_scores(features, src64[ids], dst64[ids])
    if spill_ids.size:
        out[spill_ids] = _host_scores(features, src64[spill_ids], dst64[spill_ids])
    return out


# revision 37
# speedup vs baseline: 1.0723x; 1.0226x over previous
"""DotProductPredictor edge-score kernel for 8 TRN2 NeuronCores.

score[e] = sigmoid(dot(features[src[e]], features[dst[e]]))

Strategy (self-contained; shapes hardcoded):
  - Shard the 1.2M edges across 8 cores with a balanced global deal: edges
    are bucketed into 16 (src_bucket, dst_bucket) groups (4 ranges of 25000
    node ids each; dma_gather's int16 indices only address <=32768 rows) and
    each group's edges are dealt round-robin over the 8 cores, so every
    (core, group) cell has nearly identical size and one padded capacity G
    lets all 8 cores share a single compiled program.
  - features are cast to fp16 on host and laid out as a 256B-strided padded
    table [100000, 128] (64 fp16 payload + 64 zeros). The GPSIMD dma_gather
    ucode encodes the row stride in 256B units but the element size in
    bytes, so each gather descriptor moves only the 128B payload - half the
    HBM traffic of the f32 version. (bass' dma_gather wrapper over-asserts
    elem_size_bytes % 256 == 0; that restriction only applies to the
    transpose path in the ucode, so _dma_gather_raw below mirrors the
    wrapper without it.)
  - On device per group: gather src rows and dst rows (multi-packet SWDGE,
    one 9472-index instruction each; group g runs on SWDGE queue g%4 so the
    4 Q7 queue-pairs generate descriptors concurrently - measured 3.6x
    faster than one queue; per-group valid counts come from registers so
    padding costs nothing), multiply elementwise on the vector engine (fp16
    2x packed mode), segmented reduce_sum into an f32 accumulator, sigmoid
    per group on the scalar engine, and write each group's fp16 score block
    back as it completes. 6 h-tile buffers pipeline deep enough to hide the
    DVE work and buffer-release stalls under the gathers (bufs=3 measured
    +220us/pass vs 4; 5-6 another ~-35us).
  - Measured cost structure (paired repeat-slope on HW): the kernel is
    bound by per-descriptor work on the Q7 pairs (~9ns/descriptor/pair:
    index read + descriptor gen), NOT by HBM bytes - halving descriptors
    halved time while doubling bytes cost only +15%. Hence: one descriptor
    per edge endpoint is the floor of this design, and the fp16 row payload
    (vs f32) mostly helps by halving SBUF pressure (enabling bufs=4) and
    the table upload, not via bandwidth.
  - Host unpads/unsorts scores back to original edge order. Group overflow
    (statistically ~never: capacity is ~3 sigma above the max cell size)
    is computed on host as a fallback, as is any device failure.
"""
import numpy as np

N_NODES = 100000
N_EDGES = 1200000
D = 64
NC = 8
BUCKET = 25000               # node-id range per bucket (4 * 25000 = 100000)
NB = 4
NGRP = NB * NB               # 16 groups
G = 9472                     # padded edges per (core, group); mean 9375, sigma ~33
T = NGRP * G                 # 151552 padded edges per core
PADROW = 128                 # fp16 elements per padded table row (256B stride)
SCRATCH = 65536              # dynamic DMA scratch bytes/partition (descriptor rings)
NQDEF = 4                    # SWDGE queues; group g runs on queue g % NQDEF

_CACHE = {}


def _dma_gather_raw(eng, out_ap, in_ap, idxs_ap, num_idxs, num_idxs_reg,
                    elem_size, elem_step, queue_num, single_packet=False):
    """nc.gpsimd.dma_gather for the non-transpose HBM path, without the
    elem_size_bytes % 256 assert (ucode only requires that for transpose;
    the row *stride* is what must be a multiple of 256B)."""
    from concourse import mybir
    from concourse import ap_utils
    from concourse.bass import MemorySpace

    eng._assert_queue_num(queue_num)
    assert idxs_ap.dtype == mybir.dt.int16
    assert in_ap.dtype == out_ap.dtype
    assert in_ap.space == MemorySpace.DRAM
    assert idxs_ap.space == MemorySpace.SBUF
    assert out_ap.space == MemorySpace.SBUF
    assert ap_utils.ap_is_contiguous(out_ap.ap[1:])
    assert ap_utils.ap_is_contiguous(idxs_ap.ap[1:])
    assert in_ap.ap[-1][1] == out_ap.ap[-1][1] == elem_size
    assert out_ap.ap[0][1] * out_ap.ap[1][1] == ((num_idxs + 127) // 128) * 128
    assert in_ap.ap[0][0] == elem_step
    stride_bytes = elem_step * mybir.dt.size(in_ap.dtype)
    stride_bytes_256 = stride_bytes // 256
    assert stride_bytes_256 * 256 == stride_bytes and 0 < stride_bytes_256 < 256

    _in_ap = eng.lower_ap_dma(in_ap, for_custom_bir_dma=True)
    _idxs_ap = eng.lower_ap(idxs_ap)
    _out_ap = eng.lower_ap(out_ap)
    return eng.add_instruction(
        mybir.InstDMAGatherAnt(
            name=eng.bass.get_next_instruction_name(),
            ins=[*_in_ap, _idxs_ap, eng.lower_val_access(eng.to_reg(num_idxs_reg))],
            outs=[_out_ap],
            transpose=False,
            num_idxs=num_idxs,
            elem_size=elem_size,
            stride_bytes_256=stride_bytes_256,
            gen_mode=0,
            single_packet=single_packet,
            queue_num=queue_num,
            sbuf_tokens_per_rank=0,
            sbuf_free_dim_per_rank=0,
            sbuf_free_dim_pad_per_rank=0,
            sbuf_byte_offset=0,
        )
    )


def _reduce_add_raw(eng, out_ap, in_ap):
    """vector.tensor_reduce(op=add, axis=X) without the low-precision-output
    guard (the DVE accumulates fine for our 8-element partial sums; the f32
    second stage restores precision)."""
    from concourse import mybir

    return eng.add_instruction(
        mybir.InstTensorReduce(
            name=f"I-{eng.bass.next_id()}",
            op=mybir.AluOpType.add,
            axis=mybir.AxisListType.X,
            ins=[eng.lower_ap(in_ap.opt(keep_dims={0, len(in_ap.shape) - 1}), opt=False)],
            outs=[eng.lower_ap(out_ap)],
            apply_absolute_value=None,
            apply_transpose=None,
            negate=None,
        )
    )


def _build_program():
    import os
    import concourse.tile as tile
    from concourse import bacc, mybir

    skip_gather = os.environ.get("KERNEL_SKIP_GATHER") == "1"
    skip_compute = os.environ.get("KERNEL_SKIP_COMPUTE") == "1"
    single_packet = os.environ.get("KERNEL_SP") == "1"
    halfdesc = os.environ.get("KERNEL_HALFDESC") == "1"  # timing probe only
    red2 = os.environ.get("KERNEL_RED2") == "1"
    nosplit = os.environ.get("KERNEL_NOSPLIT") == "1"
    deep = os.environ.get("KERNEL_DEEP") == "1"
    nq = int(os.environ.get("KERNEL_NQ", str(NQDEF)))
    hbufs = int(os.environ.get("KERNEL_BUFS", "8" if deep else "6"))
    nrep = int(os.environ.get("KERNEL_REPEAT", "1"))
    if halfdesc:
        skip_compute = True

    nc = bacc.Bacc(
        "TRN2",
        target_bir_lowering=False,
        debug=False,
        num_devices=NC,
        # SDMA drain runs ~3x ahead of Q7 descriptor generation, so the
        # rings never fill; smaller scratch frees SBUF for deeper h-tiling
        dynamic_dma_scratch_size=int(
            os.environ.get("KERNEL_SCRATCH", "32768" if deep else str(SCRATCH))
        ),
        num_swdge_queues=max(nq, 1),
    )
    gc16 = G // 16               # idx columns per group
    feat = nc.dram_tensor("featpad", [N_NODES, PADROW], mybir.dt.float16, kind="ExternalInput").ap()
    idx_s = nc.dram_tensor("idx_s", [128, T // 16], mybir.dt.int16, kind="ExternalInput").ap()
    idx_d = nc.dram_tensor("idx_d", [128, T // 16], mybir.dt.int16, kind="ExternalInput").ap()
    counts = nc.dram_tensor("counts", [1, 128], mybir.dt.int32, kind="ExternalInput").ap()
    out = nc.dram_tensor("scores", [128, T // 128], mybir.dt.float16, kind="ExternalOutput").ap()

    cols = G // 128              # 74 columns per group block

    with tile.TileContext(nc) as tc:
        with (
            tc.tile_pool(name="idx", bufs=1) as idxp,
            tc.tile_pool(name="acc", bufs=1) as accp,
            tc.tile_pool(name="h", bufs=hbufs) as hp,
            tc.tile_pool(name="gidx", bufs=hbufs) as gip,
        ):
            cnt = idxp.tile([1, 128], mybir.dt.int32, tag="cnt")
            nc.sync.dma_start(out=cnt[:], in_=counts)
            if deep:
                ia = ib = None  # per-group rotating idx tiles (see loop)
            else:
                ia = idxp.tile([128, T // 16], mybir.dt.int16, tag="ia")
                ib = idxp.tile([128, T // 16], mybir.dt.int16, tag="ib")
                if nosplit:
                    nc.sync.dma_start(out=ia[:], in_=idx_s)
                    nc.sync.dma_start(out=ib[:], in_=idx_d)
                else:
                    # per-group slices: group 0's gathers only wait for their
                    # own index columns, not the full 2x2.4MB load
                    for g in range(NGRP):
                        sl = slice(g * gc16, (g + 1) * gc16)
                        nc.sync.dma_start(out=ia[:, sl], in_=idx_s[:, sl])
                        nc.sync.dma_start(out=ib[:, sl], in_=idx_d[:, sl])
            reg = nc.gpsimd.alloc_register("cnt_g")

            acc = accp.tile([128, T // 128], mybir.dt.float32, tag="acc")
            sig = accp.tile([128, T // 128], mybir.dt.float16, tag="sig")
            if skip_compute:
                nc.vector.memset(acc[:], 0.0)

            for rep in range(nrep):
                for g in range(NGRP):
                    bs, bd = divmod(g, NB)
                    q = g % max(nq, 1)
                    hu = hp.tile([128, cols * D], mybir.dt.float16, tag="hu")
                    hv = hp.tile([128, cols * D], mybir.dt.float16, tag="hv")
                    if skip_gather:
                        nc.vector.memset(hu[:], 0.125)
                        nc.vector.memset(hv[:], 0.25)
                    elif halfdesc:
                        # timing probe: half the descriptors, same bytes
                        # (reads the full 256B padded row per index)
                        nc.gpsimd.reg_load(reg, cnt[0:1, g : g + 1])
                        _dma_gather_raw(
                            nc.gpsimd,
                            hu[:].rearrange("p (c d) -> p c d", d=PADROW),
                            feat[bs * BUCKET : (bs + 1) * BUCKET, 0:PADROW],
                            ia[:, g * gc16 : g * gc16 + (G // 2) // 16],
                            G // 2, reg, PADROW, PADROW, q, single_packet,
                        )
                        _dma_gather_raw(
                            nc.gpsimd,
                            hv[:].rearrange("p (c d) -> p c d", d=PADROW),
                            feat[bd * BUCKET : (bd + 1) * BUCKET, 0:PADROW],
                            ib[:, g * gc16 : g * gc16 + (G // 2) // 16],
                            G // 2, reg, PADROW, PADROW, q, single_packet,
                        )
                    else:
                        if deep:
                            iag = gip.tile([128, gc16], mybir.dt.int16, tag="iag")
                            ibg = gip.tile([128, gc16], mybir.dt.int16, tag="ibg")
                            sl = slice(g * gc16, (g + 1) * gc16)
                            nc.sync.dma_start(out=iag[:], in_=idx_s[:, sl])
                            nc.sync.dma_start(out=ibg[:], in_=idx_d[:, sl])
                            ia_ap = iag[:]
                            ib_ap = ibg[:]
                        else:
                            ia_ap = ia[:, g * gc16 : (g + 1) * gc16]
                            ib_ap = ib[:, g * gc16 : (g + 1) * gc16]
                        nc.gpsimd.reg_load(reg, cnt[0:1, g : g + 1])
                        _dma_gather_raw(
                            nc.gpsimd,
                            hu[:].rearrange("p (c d) -> p c d", d=D),
                            feat[bs * BUCKET : (bs + 1) * BUCKET, 0:D],
                            ia_ap,
                            G, reg, D, PADROW, q, single_packet,
                        )
                        _dma_gather_raw(
                            nc.gpsimd,
                            hv[:].rearrange("p (c d) -> p c d", d=D),
                            feat[bd * BUCKET : (bd + 1) * BUCKET, 0:D],
                            ib_ap,
                            G, reg, D, PADROW, q, single_packet,
                        )
                    if not skip_compute:
                        nc.vector.tensor_tensor(
                            out=hu[:], in0=hu[:], in1=hv[:], op=mybir.AluOpType.mult,
                        )
                        if red2:
                            # two-stage reduce keeps stage 1 all-fp16 (DVE 2x
                            # packed mode); stage 2 accumulates in f32
                            _reduce_add_raw(
                                nc.vector,
                                hv[:, : cols * 8],
                                hu[:].rearrange("p (c a b) -> p c a b", a=8, b=8),
                            )
                            nc.vector.reduce_sum(
                                out=acc[:, g * cols : (g + 1) * cols],
                                in_=hv[:, : cols * 8].rearrange("p (c a) -> p c a", a=8),
                                axis=mybir.AxisListType.X,
                            )
                        else:
                            nc.vector.reduce_sum(
                                out=acc[:, g * cols : (g + 1) * cols],
                                in_=hu[:].rearrange("p (c d) -> p c d", d=D),
                                axis=mybir.AxisListType.X,
                            )
                        if not nosplit:
                            # per-group tail: sigmoid + writeback pipeline
                            # behind the remaining groups' gathers
                            gs = slice(g * cols, (g + 1) * cols)
                            nc.scalar.activation(
                                sig[:, gs], acc[:, gs],
                                mybir.ActivationFunctionType.Sigmoid,
                            )
                            nc.sync.dma_start(out=out[:, gs], in_=sig[:, gs])

            if nosplit or skip_compute:
                nc.scalar.activation(sig[:], acc[:], mybir.ActivationFunctionType.Sigmoid)
                nc.sync.dma_start(out=out, in_=sig[:])

    nc.compile()
    return nc


def _prep_all(features, src64, dst64):
    """Host layout: deal each of the 16 groups' edges round-robin over the 8
    cores, build per-core padded bucket-local int16 index arrays (wrapped
    into dma_gather's 16-partition layout), per-(core,group) valid counts,
    and the mapping back to original edge order.

    Returns (in_maps, core_of, flatpos, spill_ids, featpad16).
      core_of[e]: which core computes edge e (-1 if spilled to host)
      flatpos[e]: padded position of edge e within its core's T-vector
    """
    featpad = np.zeros((N_NODES, PADROW), dtype=np.float16)
    featpad[:, :D] = features.astype(np.float16)

    grp = (src64 // BUCKET) * NB + (dst64 // BUCKET)     # [E]
    order = np.argsort(grp, kind="stable")
    sizes = np.bincount(grp, minlength=NGRP)
    starts = np.zeros(NGRP, dtype=np.int64)
    np.cumsum(sizes[:-1], out=starts[1:])

    core_of = np.empty(N_EDGES, dtype=np.int8)
    flatpos = np.empty(N_EDGES, dtype=np.int64)
    s_pad = np.full((NC, T), -1, dtype=np.int16)
    d_pad = np.full((NC, T), -1, dtype=np.int16)
    counts = np.zeros((NC, 128), dtype=np.int32)
    spill = []
    import os
    sort_cell = os.environ.get("KERNEL_SORT", "0") == "1"
    for g in range(NGRP):
        members = order[starts[g] : starts[g] + sizes[g]]
        j = np.arange(members.shape[0], dtype=np.int64)
        core = (j % NC).astype(np.int8)
        ok = (j // NC) < G
        if not ok.all():
            spill.append(members[~ok])
            members, core = members[ok], core[ok]
        sl = (src64[members] - (g // NB) * BUCKET).astype(np.int16)
        dl = (dst64[members] - (g % NB) * BUCKET).astype(np.int16)
        for c in range(NC):
            m = core == c
            k = int(m.sum())
            mem_c, sl_c, dl_c = members[m], sl[m], dl[m]
            if sort_cell:
                # ascending src ids -> consecutive gather descriptors hit
                # nearby HBM rows (DRAM row-buffer locality)
                o = np.argsort(sl_c, kind="stable")
                mem_c, sl_c, dl_c = mem_c[o], sl_c[o], dl_c[o]
            s_pad[c, g * G : g * G + k] = sl_c
            d_pad[c, g * G : g * G + k] = dl_c
            core_of[mem_c] = c
            flatpos[mem_c] = g * G + np.arange(k, dtype=np.int64)
            v2 = max(k, 128)
            if v2 > k:
                s_pad[c, g * G + k : g * G + v2] = 0
                d_pad[c, g * G + k : g * G + v2] = 0
            counts[c, g] = v2
    spill_ids = np.concatenate(spill) if spill else np.zeros(0, dtype=np.int64)
    core_of[spill_ids] = -1
    flatpos[spill_ids] = -1

    def wrap(arr):
        w = arr.reshape(T // 16, 16).T           # [16, T/16]
        return np.ascontiguousarray(np.tile(w, (8, 1)))  # [128, T/16]

    in_maps = []
    for c in range(NC):
        in_maps.append({
            "featpad": featpad,
            "idx_s": wrap(s_pad[c]),
            "idx_d": wrap(d_pad[c]),
            "counts": counts[c].reshape(1, 128),
        })
    return in_maps, core_of, flatpos, spill_ids, featpad


def _host_scores(features, s, d):
    sc = np.einsum("ij,ij->i", features[s], features[d], dtype=np.float32)
    return (1.0 / (1.0 + np.exp(-sc))).astype(np.float32)


def kernel(features, src, dst):
    from concourse.bass_utils import run_bass_kernel_spmd

    features = np.asarray(features, dtype=np.float32)
    src64 = np.asarray(src).astype(np.int64)
    dst64 = np.asarray(dst).astype(np.int64)

    if features.shape != (N_NODES, D) or src64.shape != (N_EDGES,) or dst64.shape != (N_EDGES,):
        return _host_scores(features, src64, dst64)

    if "nc" not in _CACHE:
        _CACHE["nc"] = _build_program()
    nc = _CACHE["nc"]

    in_maps, core_of, flatpos, spill_ids, _ = _prep_all(features, src64, dst64)

    try:
        res = run_bass_kernel_spmd(nc, in_maps, list(range(NC))).results
    except Exception:
        # device failure: fall back to a correct host computation
        return _host_scores(features, src64, dst64)

    rng = np.random.default_rng(12345)
    out = np.empty(N_EDGES, dtype=np.float32)
    eids = np.arange(N_EDGES, dtype=np.int64)
    for c in range(NC):
        scores_pad = res[c]["scores"].T.ravel().astype(np.float32)  # padded pos -> score
        m = core_of == c
        ids = eids[m]
        out[ids] = scores_pad[flatpos[ids]]
        # cheap integrity check on a random sample; recompute on host if the
        # device result is corrupt (defends against rare SWDGE ring races)
        probe = rng.choice(ids, size=min(2048, ids.size), replace=False)
        want = _host_scores(features, src64[probe], dst64[probe])
        if not np.allclose(out[probe], want, rtol=2e-2, atol=5e-3):
            out[ids] = _host_scores(features, src64[ids], dst64[ids])
    if spill_ids.size:
        out[spill_ids] = _host_scores(features, src64[spill_ids], dst64[spill_ids])
    return out
